# revision 12
# baseline (speedup 1.0000x reference)
"""Trainium2 Bass kernel: dense transformer block (bilinear attention, no softmax).

Reference computation (B=2, S=2048, C=1024, H=16 heads, hd=64, HIDDEN=1024):
    q = split_heads(x @ Wq.T + bq) * hd**-0.5
    k = split_heads(x @ Wk.T + bk)
    v = split_heads(x @ Wv.T + bv)
    out = (q @ k.T) @ v          per (batch, head)   <-- no softmax!
    h = gelu(out @ W1.T + b1);  mlp = h @ W2.T + b2
    y = x + out + mlp

Key algebraic optimization: (q @ k.T) @ v == q @ (k.T @ v). k.T@v is a tiny
[64,64] per head, so attention drops from ~34 GFLOP to ~1 GFLOP.

Sharding (8 cores): rows (batch*seq = 4096) split 512/core; cores 0-3 hold
batch 0, cores 4-7 batch 1. Each core computes q/k/v/MLP for its rows only.
The only cross-core data dependency is ktv = k.T@v (contraction over the full
2048 rows of a batch): each core computes its partial ktv [64,1024] (16 heads)
and a 256KB AllReduce over each 4-core batch group completes it. k/v are
computed first with chunked DMAs + contraction-outer loops so the AllReduce
triggers as early as possible; the q projection and all remaining weight DMAs
overlap the collective's ~25us firmware latency.

All matmuls run in bf16 with fp32 PSUM accumulation (validated ~4e-3 absmax
relative error vs the fp32 reference). Weights are pre-transposed/packed on
host so every DMA is contiguous and every matmul operand is a natural slice.
"""

import sys
import types

sys.path.insert(0, "/opt/trn_rl_repo")

import numpy as np
import ml_dtypes

# ---------------------------------------------------------------------------
# NTFF profile hook shim (this image's antenv lacks axon_hooks; inject it so
# run_bass_kernel_spmd(trace=True) can profile). Harmless when unused.
# ---------------------------------------------------------------------------
if "antenv.axon_hooks" not in sys.modules:
    _m = types.ModuleType("antenv.axon_hooks")
    _m._hook = None
    _m.set_axon_ntff_profile_hook = lambda h: setattr(_m, "_hook", h)
    _m.get_axon_ntff_profile_hook = lambda: _m._hook
    sys.modules["antenv.axon_hooks"] = _m
    try:
        import antenv

        antenv.axon_hooks = _m
        from trn_agent_boot.trn_boot import _ntff_profile_via_ctypes

        _m.set_axon_ntff_profile_hook(
            _ntff_profile_via_ctypes("/opt/axon/libaxon_pjrt.so")
        )
    except Exception:
        pass

import concourse.bass as bass
import concourse.mybir as mybir
import concourse.tile as tile
from concourse import bacc
from concourse import bass_utils

bass_utils.upload_artifacts = lambda tmpdir: tmpdir  # no fish bucket here
from concourse.bass_utils import run_bass_kernel_spmd

BF16 = mybir.dt.bfloat16
F32 = mybir.dt.float32
AF = mybir.ActivationFunctionType
ALU = mybir.AluOpType

B, S, C = 2, 2048, 1024
NH, HD = 16, 64
SCALE = HD ** -0.5
NCORES = 8
R = (B * S) // NCORES        # 512 rows per core
P = 128
CH = C // P                  # 8 contraction chunks
RCH = R // P                 # 4 row chunks per core
HP = NH // 2                 # 8 head-pairs (one 128-partition chunk each)

_CACHE = {}


def _build(kv_bias: bool):
    """Build + compile the 8-core SPMD program. Returns the Bacc graph."""
    nc = bacc.Bacc("TRN2", target_bir_lowering=False, debug=False, num_devices=NCORES)

    # ---- DRAM I/O (per-core shapes; data differs per core) ----
    xtb_d = nc.dram_tensor("xtb", [P, CH * R], BF16, kind="ExternalInput")
    wq_d = nc.dram_tensor("wq", [P, CH * C], BF16, kind="ExternalInput")
    wk_d = nc.dram_tensor("wk", [P, CH * C], BF16, kind="ExternalInput")
    wv_d = nc.dram_tensor("wv", [P, CH * C], BF16, kind="ExternalInput")
    w1_d = nc.dram_tensor("w1", [P, CH * C], BF16, kind="ExternalInput")
    w2_d = nc.dram_tensor("w2", [P, CH * C], BF16, kind="ExternalInput")
    bqs_d = nc.dram_tensor("bqs", [P, CH], F32, kind="ExternalInput")
    b1r_d = nc.dram_tensor("b1r", [P, CH], F32, kind="ExternalInput")
    b2r_d = nc.dram_tensor("b2r", [P, CH], F32, kind="ExternalInput")
    if kv_bias:
        bkr_d = nc.dram_tensor("bkr", [1, C], BF16, kind="ExternalInput")
        bvr_d = nc.dram_tensor("bvr", [1, C], BF16, kind="ExternalInput")
    yt_d = nc.dram_tensor("yt", [P, CH * R], F32, kind="ExternalOutput")

    # Internal DRAM for the ktv AllReduce (16 heads x [64,64] = [64, 1024]).
    # NB: Shared addr_space is only supported for >4-core groups; Local output
    # is fine for this 256KB reduce.
    ktv_loc = [nc.dram_tensor(f"ktv_loc{i}", [P, HP * P // 2], BF16) for i in (0, 1)]
    ktv_red = [nc.dram_tensor(f"ktv_red{i}", [P, HP * P // 2], BF16) for i in (0, 1)]
    groups = [[0, 1, 2, 3], [4, 5, 6, 7]]

    with tile.TileContext(nc) as tc:
        with (
            tc.tile_pool(name="persist", bufs=1) as pp,
            tc.tile_pool(name="ypool", bufs=3) as yp,
            tc.tile_pool(name="psum", bufs=8, space="PSUM") as psp,
        ):
            # ---- persistent SBUF tiles ----
            # x' and Wk/Wv are chunked per contraction block so the k/v
            # matmul pipeline starts as soon as the first chunks land.
            xtb = [pp.tile([P, R], BF16, name=f"xtb{c}") for c in range(CH)]
            wk = [pp.tile([P, C], BF16, name=f"wk{c}") for c in range(CH)]
            wv = [pp.tile([P, C], BF16, name=f"wv{c}") for c in range(CH)]
            wq = pp.tile([P, CH * C], BF16, name="wq_sb")
            w1 = pp.tile([P, CH * C], BF16, name="w1_sb")
            w2 = pp.tile([P, CH * C], BF16, name="w2_sb")
            bqs = pp.tile([P, CH], F32, name="bqs_sb")
            b1r = pp.tile([P, CH], F32, name="b1r_sb")
            b2r = pp.tile([P, CH], F32, name="b2r_sb")
            k_sb = [pp.tile([P, C], BF16, name=f"k_sb{i}") for i in range(RCH)]
            v_sb = [pp.tile([P, C], BF16, name=f"v_sb{i}") for i in range(RCH)]
            q_sb = [pp.tile([P, R], BF16, name=f"q_sb{i}") for i in range(HP)]
            out_f = [pp.tile([P, R], F32, name=f"out_f{i}") for i in range(HP)]
            out_b = [pp.tile([P, R], BF16, name=f"out_b{i}") for i in range(HP)]
            h_sb = [pp.tile([P, R], BF16, name=f"h_sb{i}") for i in range(HP)]
            ktv_acc = [
                pp.tile([P, HP * P // 2], BF16, name=f"ktv_acc{i}") for i in (0, 1)
            ]
            ktv_bb = pp.tile([P, HP * P], BF16, name="ktv_bb")
            if kv_bias:
                ones = pp.tile([1, P], BF16, name="ones_sb")
                bkr = pp.tile([1, C], BF16, name="bkr_sb")
                bvr = pp.tile([1, C], BF16, name="bvr_sb")

            # ---- input DMAs (sync engine; in exact need-order) ----
            for c in range(CH):
                nc.sync.dma_start(out=xtb[c][:], in_=xtb_d[:, c * R : (c + 1) * R])
                nc.sync.dma_start(out=wk[c][:], in_=wk_d[:, c * C : (c + 1) * C])
            for c in range(CH):
                nc.sync.dma_start(out=wv[c][:], in_=wv_d[:, c * C : (c + 1) * C])
            if kv_bias:
                nc.vector.memset(ones[:], 1.0)
                nc.sync.dma_start(out=bkr[:], in_=bkr_d[:])
                nc.sync.dma_start(out=bvr[:], in_=bvr_d[:])
            nc.sync.dma_start(out=wq[:], in_=wq_d[:])
            nc.sync.dma_start(out=bqs[:], in_=bqs_d[:])
            nc.sync.dma_start(out=w1[:], in_=w1_d[:])
            nc.sync.dma_start(out=b1r[:], in_=b1r_d[:])
            nc.sync.dma_start(out=w2[:], in_=w2_d[:])
            nc.sync.dma_start(out=b2r[:], in_=b2r_d[:])
            # zero the ktv block-diagonal staging tiles early (the zeros ride
            # through the AllReduce, so ktv_bb needs no memset)
            nc.vector.memset(ktv_acc[0][:], 0.0)
            nc.vector.memset(ktv_acc[1][:], 0.0)

            # PE warm-up: ~4us of tiny matmuls during the DMA lead-in so the
            # HAM clock gate reaches full rate before the real work arrives.
            warm = pp.tile([1, HD], BF16, name="warm_sb")
            nc.vector.memset(warm[:], 0.0)
            pw = psp.tile([HD, HD], F32, name="ps", tag="ps")
            for i in range(60):
                nc.tensor.matmul(pw[:], warm[:1, :], warm[:1, :],
                                 start=(i == 0), stop=(i == 59))

            # ---- k, v projections (row-major [r, o]) ----
            # contraction-OUTER loops, split by output half (oh): compute
            # k(oh) then v(oh), then the 4 head-pair ktv blocks of that half,
            # and launch that half's AllReduce immediately. The first
            # collective's firmware latency overlaps the second half's
            # matmuls; the second's overlaps the q projection.
            def proj_kv(w_c, brow, dst, oh):
                pss = [
                    psp.tile([P, 512], F32, name="ps", tag="ps")
                    for _ in range(RCH)
                ]
                for c in range(CH):
                    for ri in range(RCH):
                        nc.tensor.matmul(
                            pss[ri][:],
                            xtb[c][:, ri * P : (ri + 1) * P],
                            w_c[c][:, oh * 512 : (oh + 1) * 512],
                            start=(c == 0),
                            stop=(c == CH - 1 and not kv_bias),
                        )
                for ri in range(RCH):
                    ps = pss[ri]
                    if kv_bias:
                        nc.tensor.matmul(
                            ps[:],
                            ones[:1, :],
                            brow[:1, oh * 512 : (oh + 1) * 512],
                            start=False,
                            stop=True,
                        )
                    dst_ap = dst[ri][:, oh * 512 : (oh + 1) * 512]
                    if ri % 2 == 0:
                        nc.vector.tensor_copy(dst_ap, ps[:])
                    else:
                        nc.scalar.activation(dst_ap, ps[:], AF.Copy)

            for oh in range(2):
                proj_kv(wk, bkr if kv_bias else None, k_sb, oh)
                proj_kv(wv, bvr if kv_bias else None, v_sb, oh)

                # partial ktv for this half: head-pairs packed [128,128].
                # psum block for pair hp: [0:64,0:64] = ktv(2hp),
                # [64:128,64:128] = ktv(2hp+1); off-diagonal is garbage.
                # Evict the two diagonal blocks straight into the
                # block-diagonal staging layout (zeros pre-memset).
                with tc.high_priority(offset=400):
                    for hpl in range(HP // 2):
                        hp = oh * (HP // 2) + hpl
                        pk = psp.tile([P, P], F32, name="ps", tag="ps")
                        for ri in range(RCH):
                            nc.tensor.matmul(
                                pk[:],
                                k_sb[ri][:, hp * P : (hp + 1) * P],
                                v_sb[ri][:, hp * P : (hp + 1) * P],
                                start=(ri == 0),
                                stop=(ri == RCH - 1),
                            )
                        nc.vector.tensor_copy(
                            ktv_acc[oh][0:HD, hpl * P : hpl * P + HD], pk[0:HD, 0:HD]
                        )
                        nc.vector.tensor_copy(
                            ktv_acc[oh][HD:P, hpl * P + HD : (hpl + 1) * P],
                            pk[HD:P, HD:P],
                        )
                with tc.high_priority():
                    nc.scalar.dma_start(out=ktv_loc[oh][:], in_=ktv_acc[oh][:])
                    nc.gpsimd.collective_compute(
                        "AllReduce",
                        ALU.add,
                        replica_groups=groups,
                        ins=[ktv_loc[oh][:]],
                        outs=[ktv_red[oh][:]],
                    )

            # ---- q' projection (feature-major [o, r]), overlaps AllReduce ----
            for m in range(CH):
                ps = psp.tile([P, 512], F32, name="ps", tag="ps")
                for c in range(CH):
                    nc.tensor.matmul(
                        ps[:],
                        wq[:, c * C + m * P : c * C + (m + 1) * P],
                        xtb[c][:],
                        start=(c == 0),
                        stop=(c == CH - 1),
                    )
                nc.scalar.activation(
                    q_sb[m][:], ps[:], AF.Identity, bias=bqs[:, m : m + 1]
                )

            # ---- out' = blockdiag(ktv).T @ q', interleaved with MLP ----
            # The reduced halves arrive in block-diagonal layout; one verbatim
            # DMA each. After the first half's out' chunks, start the h'
            # contraction partially (o-chunks 0-3, j-groups 0-5) to overlap
            # the second collective; finish h' once the second half lands.
            HPH = HP // 2
            hps = []

            def out_chunk(hp):
                ps = psp.tile([P, 512], F32, name="ps", tag="ps")
                nc.tensor.matmul(
                    ps[:],
                    ktv_bb[:, hp * P : (hp + 1) * P],
                    q_sb[hp][:],
                    start=True,
                    stop=True,
                )
                nc.vector.tensor_copy(out_f[hp][:], ps[:])
                nc.scalar.activation(out_b[hp][:], ps[:], AF.Copy)
                # pre-add residual x into out_f (frees the tail)
                nc.vector.tensor_add(out_f[hp][:], out_f[hp][:], xtb[hp][:])

            with tc.high_priority(offset=200):
                nc.sync.dma_start(out=ktv_bb[:, 0 : HPH * P], in_=ktv_red[0][:])
            for hp in range(HPH):
                out_chunk(hp)
            # h' partial: j-groups 0-5 over the available o-chunks 0-3
            for j in range(6):
                ps = psp.tile([P, 512], F32, name="ps", tag="ps")
                hps.append(ps)
                for o in range(4):
                    nc.tensor.matmul(
                        ps[:],
                        w1[:, o * C + j * P : o * C + (j + 1) * P],
                        out_b[o][:],
                        start=(o == 0),
                        stop=False,
                    )
            with tc.high_priority(offset=200):
                nc.sync.dma_start(out=ktv_bb[:, HPH * P : HP * P], in_=ktv_red[1][:])
            for hp in range(HPH, HP):
                out_chunk(hp)

            # ---- MLP hidden: h' = gelu(W1 out' + b1) (finish) ----
            for j in range(6):
                ps = hps[j]
                for o in range(4, CH):
                    nc.tensor.matmul(
                        ps[:],
                        w1[:, o * C + j * P : o * C + (j + 1) * P],
                        out_b[o][:],
                        start=False,
                        stop=(o == CH - 1),
                    )
                nc.scalar.activation(
                    h_sb[j][:], ps[:], AF.Gelu, bias=b1r[:, j : j + 1]
                )
            for j in range(6, CH):
                ps = psp.tile([P, 512], F32, name="ps", tag="ps")
                for o in range(CH):
                    nc.tensor.matmul(
                        ps[:],
                        w1[:, o * C + j * P : o * C + (j + 1) * P],
                        out_b[o][:],
                        start=(o == 0),
                        stop=(o == CH - 1),
                    )
                nc.scalar.activation(
                    h_sb[j][:], ps[:], AF.Gelu, bias=b1r[:, j : j + 1]
                )

            # ---- MLP out + residual: y' = (W2 h' + b2) + (out' + x') ----
            for m in range(CH):
                ps = psp.tile([P, 512], F32, name="ps", tag="ps")
                for j in range(CH):
                    nc.tensor.matmul(
                        ps[:],
                        w2[:, j * C + m * P : j * C + (m + 1) * P],
                        h_sb[j][:],
                        start=(j == 0),
                        stop=(j == CH - 1),
                    )
                y_t = yp.tile([P, 512], F32, name="y_t")
                nc.vector.scalar_tensor_tensor(
                    y_t[:],
                    ps[:],
                    b2r[:, m : m + 1],
                    out_f[m][:],
                    ALU.add,
                    ALU.add,
                )
                nc.sync.dma_start(out=yt_d[:, m * R : (m + 1) * R], in_=y_t[:])

    nc.compile()
    return nc


def _get_nc(kv_bias: bool):
    key = ("nc", kv_bias)
    if key not in _CACHE:
        _CACHE[key] = _build(kv_bias)
    return _CACHE[key]


def _pack_pf(a):
    """[CH*P, F] row-major -> [P, CH*F] (partition-chunk packing)."""
    n, f = a.shape
    ch = n // P
    return np.ascontiguousarray(a.reshape(ch, P, f).transpose(1, 0, 2).reshape(P, ch * f))


def _prep_inputs(x, Wq, bq, Wk, bk, Wv, bv, W1, b1, W2, b2, kv_bias):
    bf = ml_dtypes.bfloat16
    wq_p = _pack_pf((Wq.T * SCALE).astype(np.float32)).astype(bf)
    wk_p = _pack_pf(np.ascontiguousarray(Wk.T)).astype(bf)
    wv_p = _pack_pf(np.ascontiguousarray(Wv.T)).astype(bf)
    w1_p = _pack_pf(np.ascontiguousarray(W1.T)).astype(bf)
    w2_p = _pack_pf(np.ascontiguousarray(W2.T)).astype(bf)
    bqs = np.ascontiguousarray((bq * SCALE).astype(np.float32).reshape(CH, P).T)
    b1r = np.ascontiguousarray(b1.astype(np.float32).reshape(CH, P).T)
    b2r = np.ascontiguousarray(b2.astype(np.float32).reshape(CH, P).T)

    xf = x.reshape(B * S, C)
    in_maps = []
    for core in range(NCORES):
        xs = xf[core * R : (core + 1) * R]           # [R, C]
        xt = _pack_pf(np.ascontiguousarray(xs.T))    # [P, CH*R] f32
        m = {
            "xtb": xt.astype(bf),
            "wq": wq_p,
            "wk": wk_p,
            "wv": wv_p,
            "w1": w1_p,
            "w2": w2_p,
            "bqs": bqs,
            "b1r": b1r,
            "b2r": b2r,
        }
        if kv_bias:
            m["bkr"] = bk.astype(bf).reshape(1, C)
            m["bvr"] = bv.astype(bf).reshape(1, C)
        in_maps.append(m)
    return in_maps


def _unpack_out(results):
    y = np.empty((B * S, C), np.float32)
    for core in range(NCORES):
        yt = results[core]["yt"]                     # [P, CH*R]
        blk = yt.reshape(P, CH, R).transpose(1, 0, 2).reshape(C, R)
        y[core * R : (core + 1) * R] = blk.T
    return y.reshape(B, S, C)


def _run(inputs, trace=False, trace_cores=None):
    x = np.asarray(inputs["x"], np.float32)
    args = [np.asarray(inputs[k], np.float32) for k in
            ("Wq", "bq", "Wk", "bk", "Wv", "bv", "W1", "b1", "W2", "b2")]
    kv_bias = bool(np.any(args[3]) or np.any(args[5]))
    nc = _get_nc(kv_bias)
    in_maps = _prep_inputs(x, *args, kv_bias)
    res = run_bass_kernel_spmd(
        nc, in_maps, core_ids=list(range(NCORES)), trace=trace,
        trace_cores=trace_cores,
    )
    return _unpack_out(res.results), res


def kernel(**inputs) -> np.ndarray:
    out, _ = _run(inputs, trace=False)
    return out


def kernel_profiled(**inputs):
    """Returns (output, exec_time_ns) using neuron-profile NTFF timing."""
    out, res = _run(inputs, trace=True)
    return out, res.exec_time_ns


# revision 13
# speedup vs baseline: 1.0370x; 1.0370x over previous
"""Trainium2 Bass kernel: dense transformer block (bilinear attention, no softmax).

Reference computation (B=2, S=2048, C=1024, H=16 heads, hd=64, HIDDEN=1024):
    q = split_heads(x @ Wq.T + bq) * hd**-0.5
    k = split_heads(x @ Wk.T + bk)
    v = split_heads(x @ Wv.T + bv)
    out = (q @ k.T) @ v          per (batch, head)   <-- no softmax!
    h = gelu(out @ W1.T + b1);  mlp = h @ W2.T + b2
    y = x + out + mlp

Key algebraic optimization: (q @ k.T) @ v == q @ (k.T @ v). k.T@v is a tiny
[64,64] per head, so attention drops from ~34 GFLOP to ~1 GFLOP.

Sharding (8 cores): rows (batch*seq = 4096) split 512/core; cores 0-3 hold
batch 0, cores 4-7 batch 1. Each core computes q/k/v/MLP for its rows only.
The only cross-core data dependency is ktv = k.T@v (contraction over the full
2048 rows of a batch): each core computes its partial ktv [64,1024] (16 heads)
and a 256KB AllReduce over each 4-core batch group completes it. k/v are
computed first with chunked DMAs + contraction-outer loops so the AllReduce
triggers as early as possible; the q projection and all remaining weight DMAs
overlap the collective's ~25us firmware latency.

All matmuls run in bf16 with fp32 PSUM accumulation (validated ~4e-3 absmax
relative error vs the fp32 reference). Weights are pre-transposed/packed on
host so every DMA is contiguous and every matmul operand is a natural slice.
"""

import sys
import types

sys.path.insert(0, "/opt/trn_rl_repo")

import numpy as np
import ml_dtypes

# ---------------------------------------------------------------------------
# NTFF profile hook shim (this image's antenv lacks axon_hooks; inject it so
# run_bass_kernel_spmd(trace=True) can profile). Harmless when unused.
# ---------------------------------------------------------------------------
if "antenv.axon_hooks" not in sys.modules:
    _m = types.ModuleType("antenv.axon_hooks")
    _m._hook = None
    _m.set_axon_ntff_profile_hook = lambda h: setattr(_m, "_hook", h)
    _m.get_axon_ntff_profile_hook = lambda: _m._hook
    sys.modules["antenv.axon_hooks"] = _m
    try:
        import antenv

        antenv.axon_hooks = _m
        from trn_agent_boot.trn_boot import _ntff_profile_via_ctypes

        _m.set_axon_ntff_profile_hook(
            _ntff_profile_via_ctypes("/opt/axon/libaxon_pjrt.so")
        )
    except Exception:
        pass

import concourse.bass as bass
import concourse.mybir as mybir
import concourse.tile as tile
from concourse import bacc
from concourse import bass_utils

bass_utils.upload_artifacts = lambda tmpdir: tmpdir  # no fish bucket here
from concourse.bass_utils import run_bass_kernel_spmd

BF16 = mybir.dt.bfloat16
F32 = mybir.dt.float32
AF = mybir.ActivationFunctionType
ALU = mybir.AluOpType

B, S, C = 2, 2048, 1024
NH, HD = 16, 64
SCALE = HD ** -0.5
NCORES = 8
R = (B * S) // NCORES        # 512 rows per core
P = 128
CH = C // P                  # 8 contraction chunks
RCH = R // P                 # 4 row chunks per core
HP = NH // 2                 # 8 head-pairs (one 128-partition chunk each)

_CACHE = {}


def _build(kv_bias: bool):
    """Build + compile the 8-core SPMD program. Returns the Bacc graph."""
    nc = bacc.Bacc("TRN2", target_bir_lowering=False, debug=False, num_devices=NCORES)

    # ---- DRAM I/O (per-core shapes; data differs per core) ----
    xtb_d = nc.dram_tensor("xtb", [P, CH * R], BF16, kind="ExternalInput")
    wq_d = nc.dram_tensor("wq", [P, CH * C], BF16, kind="ExternalInput")
    wk_d = nc.dram_tensor("wk", [P, CH * C], BF16, kind="ExternalInput")
    wv_d = nc.dram_tensor("wv", [P, CH * C], BF16, kind="ExternalInput")
    w1_d = nc.dram_tensor("w1", [P, CH * C], BF16, kind="ExternalInput")
    w2_d = nc.dram_tensor("w2", [P, CH * C], BF16, kind="ExternalInput")
    bqs_d = nc.dram_tensor("bqs", [P, CH], F32, kind="ExternalInput")
    b1r_d = nc.dram_tensor("b1r", [P, CH], F32, kind="ExternalInput")
    b2r_d = nc.dram_tensor("b2r", [P, CH], F32, kind="ExternalInput")
    if kv_bias:
        bkr_d = nc.dram_tensor("bkr", [1, C], BF16, kind="ExternalInput")
        bvr_d = nc.dram_tensor("bvr", [1, C], BF16, kind="ExternalInput")
    yt_d = nc.dram_tensor("yt", [P, CH * R], F32, kind="ExternalOutput")

    # Internal DRAM for the ktv AllReduce (16 heads x [64,64] = [64, 1024]).
    # NB: Shared addr_space is only supported for >4-core groups; Local output
    # is fine for this 256KB reduce.
    ktv_loc = [nc.dram_tensor(f"ktv_loc{i}", [P, HP * P // 2], BF16) for i in (0, 1)]
    ktv_red = [nc.dram_tensor(f"ktv_red{i}", [P, HP * P // 2], BF16) for i in (0, 1)]
    groups = [[0, 1, 2, 3], [4, 5, 6, 7]]

    with tile.TileContext(nc) as tc:
        with (
            tc.tile_pool(name="persist", bufs=1) as pp,
            tc.tile_pool(name="ypool", bufs=3) as yp,
            tc.tile_pool(name="psum", bufs=8, space="PSUM") as psp,
        ):
            # ---- persistent SBUF tiles ----
            # x' and Wk/Wv are chunked per contraction block so the k/v
            # matmul pipeline starts as soon as the first chunks land.
            xtb = [pp.tile([P, R], BF16, name=f"xtb{c}") for c in range(CH)]
            wk = [pp.tile([P, C], BF16, name=f"wk{c}") for c in range(CH)]
            wv = [pp.tile([P, C], BF16, name=f"wv{c}") for c in range(CH)]
            wq = pp.tile([P, CH * C], BF16, name="wq_sb")
            w1 = pp.tile([P, CH * C], BF16, name="w1_sb")
            w2 = pp.tile([P, CH * C], BF16, name="w2_sb")
            bqs = pp.tile([P, CH], F32, name="bqs_sb")
            b1r = pp.tile([P, CH], F32, name="b1r_sb")
            b2r = pp.tile([P, CH], F32, name="b2r_sb")
            k_sb = [pp.tile([P, C], BF16, name=f"k_sb{i}") for i in range(RCH)]
            v_sb = [pp.tile([P, C], BF16, name=f"v_sb{i}") for i in range(RCH)]
            q_sb = [pp.tile([P, R], BF16, name=f"q_sb{i}") for i in range(HP)]
            out_f = [pp.tile([P, R], F32, name=f"out_f{i}") for i in range(HP)]
            out_b = [pp.tile([P, R], BF16, name=f"out_b{i}") for i in range(HP)]
            h_sb = [pp.tile([P, R], BF16, name=f"h_sb{i}") for i in range(HP)]
            ktv_acc = [
                pp.tile([P, HP * P // 2], BF16, name=f"ktv_acc{i}") for i in (0, 1)
            ]
            ktv_bb = pp.tile([P, HP * P], BF16, name="ktv_bb")
            if kv_bias:
                ones = pp.tile([1, P], BF16, name="ones_sb")
                bkr = pp.tile([1, C], BF16, name="bkr_sb")
                bvr = pp.tile([1, C], BF16, name="bvr_sb")

            # ---- input DMAs (sync engine; in exact need-order) ----
            for c in range(CH):
                nc.sync.dma_start(out=xtb[c][:], in_=xtb_d[:, c * R : (c + 1) * R])
                nc.sync.dma_start(out=wk[c][:], in_=wk_d[:, c * C : (c + 1) * C])
            for c in range(CH):
                nc.sync.dma_start(out=wv[c][:], in_=wv_d[:, c * C : (c + 1) * C])
            if kv_bias:
                nc.vector.memset(ones[:], 1.0)
                nc.sync.dma_start(out=bkr[:], in_=bkr_d[:])
                nc.sync.dma_start(out=bvr[:], in_=bvr_d[:])
            nc.sync.dma_start(out=wq[:], in_=wq_d[:])
            nc.sync.dma_start(out=bqs[:], in_=bqs_d[:])
            nc.sync.dma_start(out=w1[:], in_=w1_d[:])
            nc.sync.dma_start(out=b1r[:], in_=b1r_d[:])
            nc.sync.dma_start(out=w2[:], in_=w2_d[:])
            nc.sync.dma_start(out=b2r[:], in_=b2r_d[:])
            # zero the ktv block-diagonal staging tiles early (the zeros ride
            # through the AllReduce, so ktv_bb needs no memset)
            nc.vector.memset(ktv_acc[0][:], 0.0)
            nc.vector.memset(ktv_acc[1][:], 0.0)

            # PE warm-up: ~4us of tiny matmuls during the DMA lead-in so the
            # HAM clock gate reaches full rate before the real work arrives.
            warm = pp.tile([1, HD], BF16, name="warm_sb")
            nc.vector.memset(warm[:], 0.0)
            pw = psp.tile([HD, HD], F32, name="ps", tag="ps")
            for i in range(60):
                nc.tensor.matmul(pw[:], warm[:1, :], warm[:1, :],
                                 start=(i == 0), stop=(i == 59))

            # ---- k, v projections (row-major [r, o]) ----
            # contraction-OUTER loops, split by output half (oh): compute
            # k(oh) then v(oh), then the 4 head-pair ktv blocks of that half,
            # and launch that half's AllReduce immediately. The first
            # collective's firmware latency overlaps the second half's
            # matmuls; the second's overlaps the q projection.
            def proj_kv(w_c, brow, dst, oh):
                pss = [
                    psp.tile([P, 512], F32, name="ps", tag="ps")
                    for _ in range(RCH)
                ]
                for c in range(CH):
                    for ri in range(RCH):
                        nc.tensor.matmul(
                            pss[ri][:],
                            xtb[c][:, ri * P : (ri + 1) * P],
                            w_c[c][:, oh * 512 : (oh + 1) * 512],
                            start=(c == 0),
                            stop=(c == CH - 1 and not kv_bias),
                        )
                for ri in range(RCH):
                    ps = pss[ri]
                    if kv_bias:
                        nc.tensor.matmul(
                            ps[:],
                            ones[:1, :],
                            brow[:1, oh * 512 : (oh + 1) * 512],
                            start=False,
                            stop=True,
                        )
                    dst_ap = dst[ri][:, oh * 512 : (oh + 1) * 512]
                    if ri % 2 == 0:
                        nc.vector.tensor_copy(dst_ap, ps[:])
                    else:
                        nc.scalar.activation(dst_ap, ps[:], AF.Copy)

            for oh in range(2):
                proj_kv(wk, bkr if kv_bias else None, k_sb, oh)
                proj_kv(wv, bvr if kv_bias else None, v_sb, oh)

                # partial ktv for this half: head-pairs packed [128,128].
                # psum block for pair hp: [0:64,0:64] = ktv(2hp),
                # [64:128,64:128] = ktv(2hp+1); off-diagonal is garbage.
                # Evict the two diagonal blocks straight into the
                # block-diagonal staging layout (zeros pre-memset).
                with tc.high_priority(offset=400):
                    for hpl in range(HP // 2):
                        hp = oh * (HP // 2) + hpl
                        pk = psp.tile([P, P], F32, name="ps", tag="ps")
                        for ri in range(RCH):
                            nc.tensor.matmul(
                                pk[:],
                                k_sb[ri][:, hp * P : (hp + 1) * P],
                                v_sb[ri][:, hp * P : (hp + 1) * P],
                                start=(ri == 0),
                                stop=(ri == RCH - 1),
                            )
                        nc.vector.tensor_copy(
                            ktv_acc[oh][0:HD, hpl * P : hpl * P + HD], pk[0:HD, 0:HD]
                        )
                        nc.vector.tensor_copy(
                            ktv_acc[oh][HD:P, hpl * P + HD : (hpl + 1) * P],
                            pk[HD:P, HD:P],
                        )
                with tc.high_priority():
                    nc.gpsimd.dma_start(out=ktv_loc[oh][:], in_=ktv_acc[oh][:])
                    nc.gpsimd.collective_compute(
                        "AllReduce",
                        ALU.add,
                        replica_groups=groups,
                        ins=[ktv_loc[oh][:]],
                        outs=[ktv_red[oh][:]],
                    )

            # ---- q' projection (feature-major [o, r]), overlaps AllReduce ----
            for m in range(CH):
                ps = psp.tile([P, 512], F32, name="ps", tag="ps")
                for c in range(CH):
                    nc.tensor.matmul(
                        ps[:],
                        wq[:, c * C + m * P : c * C + (m + 1) * P],
                        xtb[c][:],
                        start=(c == 0),
                        stop=(c == CH - 1),
                    )
                nc.scalar.activation(
                    q_sb[m][:], ps[:], AF.Identity, bias=bqs[:, m : m + 1]
                )

            # ---- out' = blockdiag(ktv).T @ q', interleaved with MLP ----
            # The reduced halves arrive in block-diagonal layout; one verbatim
            # DMA each. After the first half's out' chunks, start the h'
            # contraction partially (o-chunks 0-3, j-groups 0-5) to overlap
            # the second collective; finish h' once the second half lands.
            HPH = HP // 2
            hps = []

            def out_chunk(hp):
                ps = psp.tile([P, 512], F32, name="ps", tag="ps")
                nc.tensor.matmul(
                    ps[:],
                    ktv_bb[:, hp * P : (hp + 1) * P],
                    q_sb[hp][:],
                    start=True,
                    stop=True,
                )
                nc.vector.tensor_copy(out_f[hp][:], ps[:])
                nc.scalar.activation(out_b[hp][:], ps[:], AF.Copy)
                # pre-add residual x into out_f (frees the tail)
                nc.vector.tensor_add(out_f[hp][:], out_f[hp][:], xtb[hp][:])

            with tc.high_priority(offset=200):
                nc.sync.dma_start(out=ktv_bb[:, 0 : HPH * P], in_=ktv_red[0][:])
            for hp in range(HPH):
                out_chunk(hp)
            # h' partial: j-groups 0-5 over the available o-chunks 0-3
            for j in range(6):
                ps = psp.tile([P, 512], F32, name="ps", tag="ps")
                hps.append(ps)
                for o in range(4):
                    nc.tensor.matmul(
                        ps[:],
                        w1[:, o * C + j * P : o * C + (j + 1) * P],
                        out_b[o][:],
                        start=(o == 0),
                        stop=False,
                    )
            with tc.high_priority(offset=200):
                nc.sync.dma_start(out=ktv_bb[:, HPH * P : HP * P], in_=ktv_red[1][:])
            for hp in range(HPH, HP):
                out_chunk(hp)

            # ---- MLP hidden: h' = gelu(W1 out' + b1) (finish) ----
            for j in range(6):
                ps = hps[j]
                for o in range(4, CH):
                    nc.tensor.matmul(
                        ps[:],
                        w1[:, o * C + j * P : o * C + (j + 1) * P],
                        out_b[o][:],
                        start=False,
                        stop=(o == CH - 1),
                    )
                nc.scalar.activation(
                    h_sb[j][:], ps[:], AF.Gelu, bias=b1r[:, j : j + 1]
                )
            for j in range(6, CH):
                ps = psp.tile([P, 512], F32, name="ps", tag="ps")
                for o in range(CH):
                    nc.tensor.matmul(
                        ps[:],
                        w1[:, o * C + j * P : o * C + (j + 1) * P],
                        out_b[o][:],
                        start=(o == 0),
                        stop=(o == CH - 1),
                    )
                nc.scalar.activation(
                    h_sb[j][:], ps[:], AF.Gelu, bias=b1r[:, j : j + 1]
                )

            # ---- MLP out + residual: y' = (W2 h' + b2) + (out' + x') ----
            for m in range(CH):
                ps = psp.tile([P, 512], F32, name="ps", tag="ps")
                for j in range(CH):
                    nc.tensor.matmul(
                        ps[:],
                        w2[:, j * C + m * P : j * C + (m + 1) * P],
                        h_sb[j][:],
                        start=(j == 0),
                        stop=(j == CH - 1),
                    )
                y_t = yp.tile([P, 512], F32, name="y_t")
                nc.vector.scalar_tensor_tensor(
                    y_t[:],
                    ps[:],
                    b2r[:, m : m + 1],
                    out_f[m][:],
                    ALU.add,
                    ALU.add,
                )
                nc.sync.dma_start(out=yt_d[:, m * R : (m + 1) * R], in_=y_t[:])

    nc.compile()
    return nc


def _get_nc(kv_bias: bool):
    key = ("nc", kv_bias)
    if key not in _CACHE:
        _CACHE[key] = _build(kv_bias)
    return _CACHE[key]


def _pack_pf(a):
    """[CH*P, F] row-major -> [P, CH*F] (partition-chunk packing)."""
    n, f = a.shape
    ch = n // P
    return np.ascontiguousarray(a.reshape(ch, P, f).transpose(1, 0, 2).reshape(P, ch * f))


def _prep_inputs(x, Wq, bq, Wk, bk, Wv, bv, W1, b1, W2, b2, kv_bias):
    bf = ml_dtypes.bfloat16
    wq_p = _pack_pf((Wq.T * SCALE).astype(np.float32)).astype(bf)
    wk_p = _pack_pf(np.ascontiguousarray(Wk.T)).astype(bf)
    wv_p = _pack_pf(np.ascontiguousarray(Wv.T)).astype(bf)
    w1_p = _pack_pf(np.ascontiguousarray(W1.T)).astype(bf)
    w2_p = _pack_pf(np.ascontiguousarray(W2.T)).astype(bf)
    bqs = np.ascontiguousarray((bq * SCALE).astype(np.float32).reshape(CH, P).T)
    b1r = np.ascontiguousarray(b1.astype(np.float32).reshape(CH, P).T)
    b2r = np.ascontiguousarray(b2.astype(np.float32).reshape(CH, P).T)

    xf = x.reshape(B * S, C)
    in_maps = []
    for core in range(NCORES):
        xs = xf[core * R : (core + 1) * R]           # [R, C]
        xt = _pack_pf(np.ascontiguousarray(xs.T))    # [P, CH*R] f32
        m = {
            "xtb": xt.astype(bf),
            "wq": wq_p,
            "wk": wk_p,
            "wv": wv_p,
            "w1": w1_p,
            "w2": w2_p,
            "bqs": bqs,
            "b1r": b1r,
            "b2r": b2r,
        }
        if kv_bias:
            m["bkr"] = bk.astype(bf).reshape(1, C)
            m["bvr"] = bv.astype(bf).reshape(1, C)
        in_maps.append(m)
    return in_maps


def _unpack_out(results):
    y = np.empty((B * S, C), np.float32)
    for core in range(NCORES):
        yt = results[core]["yt"]                     # [P, CH*R]
        blk = yt.reshape(P, CH, R).transpose(1, 0, 2).reshape(C, R)
        y[core * R : (core + 1) * R] = blk.T
    return y.reshape(B, S, C)


def _run(inputs, trace=False, trace_cores=None):
    x = np.asarray(inputs["x"], np.float32)
    args = [np.asarray(inputs[k], np.float32) for k in
            ("Wq", "bq", "Wk", "bk", "Wv", "bv", "W1", "b1", "W2", "b2")]
    kv_bias = bool(np.any(args[3]) or np.any(args[5]))
    nc = _get_nc(kv_bias)
    in_maps = _prep_inputs(x, *args, kv_bias)
    res = run_bass_kernel_spmd(
        nc, in_maps, core_ids=list(range(NCORES)), trace=trace,
        trace_cores=trace_cores,
    )
    return _unpack_out(res.results), res


def kernel(**inputs) -> np.ndarray:
    out, _ = _run(inputs, trace=False)
    return out


def kernel_profiled(**inputs):
    """Returns (output, exec_time_ns) using neuron-profile NTFF timing."""
    out, res = _run(inputs, trace=True)
    return out, res.exec_time_ns
